# Initial kernel scaffold
#
"""Trainium2 Bass kernel for AdaptiveRankFactorizedLSTM.

Problem: low-rank factorized LSTM, B=4096, T=20, D=H=512, rank=20.
  ih_all = (x @ wih_a) @ wih_b + b_ih            (precomputable projection)
  per step: gates = ih_t + (h @ whh_a) @ whh_b + b_hh; standard LSTM cell.

Strategy (8 NeuronCores, data-parallel over batch, 512 batch/core):
  - All on-device state is kept transposed: [feature -> partitions, batch -> free].
    The host pre-transposes each core's x shard to xT[t, f, b] so the device
    never transposes the 21MB input; the device writes hT[t, h, b] and the host
    transposes back on unshard.
  - Rank-20 structure: p1_t = wih_a^T x_t and q_t = whh_a^T h_{t-1} are [20,512]
    per step.  Both are computed into one PSUM tile, copied to SBUF u_t
    ([42,512]: rank rows of p1, q, plus two "ones" rows), and the gate
    pre-activations come from a single K=42 matmul against
    Wcat = [wih_b; whh_b; b_ih; b_hh]  (bias folded in via the ones rows).
  - Gates land transposed in PSUM [gate_dim -> partitions, batch -> free];
    ACT applies sigmoid/tanh straight out of PSUM; DVE does the cell update.
"""

import sys

if "/opt/trn_rl_repo" not in sys.path:
    sys.path.insert(0, "/opt/trn_rl_repo")

import numpy as np

import concourse.bass as bass
import concourse.mybir as mybir
from concourse.tile import TileContext
from concourse.bass_utils import run_bass_kernel_spmd

F32 = mybir.dt.float32
AF = mybir.ActivationFunctionType

NCORES = 8
B = 4096
BL = B // NCORES  # 512 batch per core
T = 20
D = 512  # input size
H = 512  # hidden size
R = 20  # rank
G4 = 4 * H  # 2048
P = 128
FC = D // P  # 4 feature chunks
HC = H // P  # 4 hidden chunks
KU = 2 * R + 2  # 42: p1 rows, q rows, two ones rows (b_ih, b_hh)


def _legalize_sync_waits(nc, limit=1):
    """This walrus build only encodes one sync-wait per instruction; move
    excess waits onto preceding NoOps on the same engine."""
    for fn in nc.m.functions:
        for bb in fn.blocks:
            new_insts = []
            for inst in bb.instructions:
                si = inst.sync_info
                if si is not None and si.on_wait and len(si.on_wait) > limit:
                    waits = list(si.on_wait)
                    keep = waits[-limit:]
                    head = waits[:-limit]
                    for i in range(0, len(head), limit):
                        nop = mybir.InstNoOp(name=f"{inst.name}-ssp{i}")
                        nop.engine = inst.engine
                        nop.sync_info = mybir.SyncInfo(
                            on_wait=head[i : i + limit], on_update=[]
                        )
                        nc.register_instruction(nop)
                        new_insts.append(nop)
                    inst.sync_info = mybir.SyncInfo(
                        on_wait=keep, on_update=list(si.on_update or [])
                    )
                new_insts.append(inst)
            bb.instructions[:] = new_insts


def build():
    nc = bass.Bass(target_bir_lowering=False)

    xT = nc.declare_dram_parameter("xT", [T, D, BL], F32, isOutput=False)
    wih_a = nc.declare_dram_parameter("wih_a", [D, R], F32, isOutput=False)
    wih_b = nc.declare_dram_parameter("wih_b", [R, G4], F32, isOutput=False)
    b_ih = nc.declare_dram_parameter("b_ih", [G4], F32, isOutput=False)
    whh_a = nc.declare_dram_parameter("whh_a", [D, R], F32, isOutput=False)
    whh_b = nc.declare_dram_parameter("whh_b", [R, G4], F32, isOutput=False)
    b_hh = nc.declare_dram_parameter("b_hh", [G4], F32, isOutput=False)
    outT = nc.declare_dram_parameter("outT", [T, H, BL], F32, isOutput=True)
    cT_out = nc.declare_dram_parameter("cT_out", [H, BL], F32, isOutput=True)

    # gate order in Wcat columns: i, f, g, o (j = gate*H + h)
    GATE_FUNCS = [AF.Sigmoid, AF.Sigmoid, AF.Tanh, AF.Sigmoid]

    with TileContext(nc) as tc:
        with (
            tc.tile_pool(name="singles", bufs=1) as singles,
            tc.tile_pool(name="xp", bufs=3) as xp,
            tc.tile_pool(name="up", bufs=2) as up,
            tc.tile_pool(name="actp", bufs=2) as actp,
            tc.tile_pool(name="dvep", bufs=2) as dvep,
            tc.tile_pool(name="hp", bufs=2) as hp,
            tc.tile_pool(name="gpsum", bufs=3, space="PSUM") as gpsum,
            tc.tile_pool(name="pqpsum", bufs=2, space="PSUM") as pqpsum,
        ):
            # ---- weights / constants ----
            wa_sb = singles.tile([P, FC, R], F32)
            nc.sync.dma_start(out=wa_sb, in_=wih_a.rearrange("(c p) r -> p c r", p=P))
            ua_sb = singles.tile([P, HC, R], F32)
            nc.sync.dma_start(out=ua_sb, in_=whh_a.rearrange("(c p) r -> p c r", p=P))

            wcat = singles.tile([KU, G4], F32)
            nc.sync.dma_start(out=wcat[0:R, :], in_=wih_b[:, :])
            nc.sync.dma_start(out=wcat[R : 2 * R, :], in_=whh_b[:, :])
            nc.sync.dma_start(out=wcat[2 * R : 2 * R + 1, :], in_=b_ih.rearrange("g -> 1 g"))
            nc.sync.dma_start(out=wcat[2 * R + 1 : 2 * R + 2, :], in_=b_hh.rearrange("g -> 1 g"))

            cT = singles.tile([P, HC, BL], F32)
            nc.vector.memset(cT, 0.0)

            h_prev = None
            for t in range(T):
                # ---- load xT_t and project to rank space (p1), plus q from h ----
                xt = xp.tile([P, FC, BL], F32, tag="xt")
                nc.sync.dma_start(out=xt, in_=xT[t].rearrange("(c p) b -> p c b", p=P))

                pq = pqpsum.tile([2 * R, BL], F32, tag="pq")
                for fc in range(FC):
                    nc.tensor.matmul(
                        pq[0:R, :],
                        lhsT=wa_sb[:, fc, :],
                        rhs=xt[:, fc, :],
                        start=(fc == 0),
                        stop=(fc == FC - 1),
                    )
                if t > 0:
                    for fc in range(HC):
                        nc.tensor.matmul(
                            pq[R : 2 * R, :],
                            lhsT=ua_sb[:, fc, :],
                            rhs=h_prev[:, fc, :],
                            start=(fc == 0),
                            stop=(fc == HC - 1),
                        )

                u_t = up.tile([KU, BL], F32, tag="u")
                if t == 0:
                    nc.vector.tensor_copy(u_t[0:R, :], pq[0:R, :])
                    nc.gpsimd.memset(u_t[R : 2 * R, :], 0.0)
                else:
                    nc.vector.tensor_copy(u_t[0 : 2 * R, :], pq)
                nc.gpsimd.memset(u_t[2 * R :, :], 1.0)

                # ---- gates: 4 gates x 2 halves, each [128 j, 1024] PSUM ----
                acts = []
                for gi in range(4):
                    a_t = actp.tile([P, HC, BL], F32, tag=f"act{gi}")
                    acts.append(a_t)
                    for half in range(2):
                        gp = gpsum.tile([P, 2 * BL], F32, tag="g")
                        for sub in range(2):
                            jc = gi * 4 + half * 2 + sub
                            nc.tensor.matmul(
                                gp[:, sub * BL : (sub + 1) * BL],
                                lhsT=wcat[:, jc * P : (jc + 1) * P],
                                rhs=u_t,
                                start=True,
                                stop=True,
                            )
                        nc.scalar.activation(
                            a_t[:, half * 2 : half * 2 + 2, :].rearrange(
                                "p c b -> p (c b)"
                            ),
                            gp,
                            GATE_FUNCS[gi],
                        )

                a_i, a_f, a_g, a_o = acts

                # ---- cell update (DVE) ----
                t1 = dvep.tile([P, HC, BL], F32, tag="t1")
                nc.vector.tensor_mul(t1, a_f, cT)
                t2 = dvep.tile([P, HC, BL], F32, tag="t2")
                nc.vector.tensor_mul(t2, a_i, a_g)
                nc.vector.tensor_add(cT, t1, t2)

                tanc = dvep.tile([P, HC, BL], F32, tag="tanc")
                nc.scalar.activation(tanc, cT, AF.Tanh)

                h_t = hp.tile([P, HC, BL], F32, tag="h")
                nc.vector.tensor_mul(h_t, a_o, tanc)

                nc.sync.dma_start(
                    out=outT[t].rearrange("(c p) b -> p c b", p=P), in_=h_t
                )
                h_prev = h_t

            nc.sync.dma_start(
                out=cT_out.rearrange("(c p) b -> p c b", p=P), in_=cT
            )

    _legalize_sync_waits(nc)
    return nc


_NC_CACHE = None


def _get_nc():
    global _NC_CACHE
    if _NC_CACHE is None:
        _NC_CACHE = build()
    return _NC_CACHE


def kernel(x, wih_a, wih_b, b_ih, whh_a, whh_b, b_hh):
    x = np.ascontiguousarray(np.asarray(x, dtype=np.float32))
    wih_a = np.asarray(wih_a, dtype=np.float32)
    wih_b = np.asarray(wih_b, dtype=np.float32)
    b_ih = np.asarray(b_ih, dtype=np.float32)
    whh_a = np.asarray(whh_a, dtype=np.float32)
    whh_b = np.asarray(whh_b, dtype=np.float32)
    b_hh = np.asarray(b_hh, dtype=np.float32)

    nc = _get_nc()
    in_maps = []
    for c in range(NCORES):
        xs = x[c * BL : (c + 1) * BL]  # [BL, T, D]
        xT_np = np.ascontiguousarray(xs.transpose(1, 2, 0))  # [T, D, BL]
        in_maps.append(
            {
                "xT": xT_np,
                "wih_a": wih_a,
                "wih_b": wih_b,
                "b_ih": b_ih,
                "whh_a": whh_a,
                "whh_b": whh_b,
                "b_hh": b_hh,
            }
        )

    res = run_bass_kernel_spmd(nc, in_maps, list(range(NCORES)))

    outs = []
    cs = []
    for c in range(NCORES):
        oT = res.results[c]["outT"]  # [T, H, BL]
        outs.append(np.ascontiguousarray(oT.transpose(2, 0, 1)))  # [BL, T, H]
        cs.append(np.ascontiguousarray(res.results[c]["cT_out"].T))  # [BL, H]
    output = np.concatenate(outs, axis=0)
    c_final = np.concatenate(cs, axis=0)
    h_final = np.ascontiguousarray(output[:, -1, :])
    return (output, (h_final, c_final))


if __name__ == "__main__":
    # quick self-run with random data
    rng = np.random.default_rng(0)
    s_in = 1.0 / np.sqrt(D)
    inputs = {
        "x": rng.standard_normal((B, T, D), dtype=np.float32),
        "wih_a": (rng.standard_normal((D, R)) * s_in).astype(np.float32),
        "wih_b": (rng.standard_normal((R, G4)) * s_in).astype(np.float32),
        "b_ih": (rng.standard_normal((G4,)) * s_in).astype(np.float32),
        "whh_a": (rng.standard_normal((D, R)) * s_in).astype(np.float32),
        "whh_b": (rng.standard_normal((R, G4)) * s_in).astype(np.float32),
        "b_hh": (rng.standard_normal((G4,)) * s_in).astype(np.float32),
    }
    out, (h, c) = kernel(**inputs)
    print("out", out.shape, out.dtype, "h", h.shape, "c", c.shape)


# revision 20
# speedup vs baseline: 1.2430x; 1.2430x over previous
"""Trainium2 Bass kernel for AdaptiveRankFactorizedLSTM.

Problem: low-rank factorized LSTM, B=4096, T=20, D=H=512, rank=20.
  ih_all = (x @ wih_a) @ wih_b + b_ih            (precomputable projection)
  per step: gates = ih_t + (h @ whh_a) @ whh_b + b_hh; standard LSTM cell.

Strategy (8 NeuronCores, data-parallel over batch, 512 batch/core):
  - All on-device state is kept transposed: [feature -> partitions, batch -> free].
    The host pre-transposes (and bf16-casts) each core's x shard to xT[t, f, b]
    so the device never transposes the input; the device writes hT[t, h, b] and
    the host transposes back on unshard.
  - Rank-20 structure: p1_t = wih_a^T x_t and q_t = whh_a^T h_{t-1} are tiny.
    Both accumulate into one PSUM bank via zero-padded 128-wide bf16
    stationaries (the PE fast-weight-load needs non-fp32, 128-col weights —
    fp32 weights load ~1 elem/cycle and dominated runtime), then one DVE copy
    into u_t.  Gate pre-activations = single K=128 matmul against
    Wcat = [wih_b; pad; whh_b; pad; b_ih; b_hh; pad] with "ones" rows in u
    providing the biases.
  - Gates land transposed in PSUM [gate_dim -> partitions, batch -> free]; ACT
    applies exact sigmoid/tanh straight out of PSUM (both live in the
    sigmoid_and_others table set); DVE does the bf16 cell update (c kept bf16;
    verified ~5e-3 rel err end-to-end).
  - The batch is split into two independent "waves" whose steps interleave:
    while wave A runs its cell-update tail (DVE+tanh), ACT processes wave B's
    gate activations — hiding the recurrent critical path.  ACT is the
    saturated engine (~85-90%); ~26M sigmoid/tanh elements per core at
    1 elem/cycle/lane @1.2GHz bound the kernel at roughly 200us.
"""

import sys

if "/opt/trn_rl_repo" not in sys.path:
    sys.path.insert(0, "/opt/trn_rl_repo")

import ml_dtypes
import numpy as np

import concourse.bass as bass
import concourse.mybir as mybir
from concourse.tile import TileContext
from concourse.bass_utils import run_bass_kernel_spmd

F32 = mybir.dt.float32
BF16 = mybir.dt.bfloat16
AF = mybir.ActivationFunctionType

NCORES = 8
B = 4096
BL = B // NCORES  # 512 batch per core
T = 20
D = 512  # input size
H = 512  # hidden size
R = 20  # rank
G4 = 4 * H  # 2048
P = 128
FC = D // P  # 4 feature chunks
HC = H // P  # 4 hidden chunks
RP = 32  # p1 block padded to 32 rows (matmul out base partition must be 0/32/64)
QE = RP + R  # 52: end of q rows
KU = 128  # u rows: 0-31 p1(pad), 32-51 q, 52-63 pad, 64-65 ones (biases), 66-127 pad
          # padded to K=128 so the PE fast-weight-load path is eligible


def _legalize_sync_waits(nc, limit=1):
    """This walrus build only encodes one sync-wait per instruction; move
    excess waits onto preceding NoOps on the same engine."""
    for fn in nc.m.functions:
        for bb in fn.blocks:
            new_insts = []
            for inst in bb.instructions:
                si = inst.sync_info
                if si is not None and si.on_wait and len(si.on_wait) > limit:
                    waits = list(si.on_wait)
                    keep = waits[-limit:]
                    head = waits[:-limit]
                    for i in range(0, len(head), limit):
                        nop = mybir.InstNoOp(name=f"{inst.name}-ssp{i}")
                        nop.engine = inst.engine
                        nop.sync_info = mybir.SyncInfo(
                            on_wait=head[i : i + limit], on_update=[]
                        )
                        nc.register_instruction(nop)
                        new_insts.append(nop)
                    inst.sync_info = mybir.SyncInfo(
                        on_wait=keep, on_update=list(si.on_update or [])
                    )
                new_insts.append(inst)
            bb.instructions[:] = new_insts


def build(reps=1, waves=2, tanc_chunks=1, qcopy_act=False, t2_gps=False, t1_gps=False, add_gps=False, merge_ucopy=True, xbufs=3, abufs=2):
    """reps>1 repeats the whole computation serially inside one NEFF (timing).
    waves: independent batch sub-recurrences interleaved per step to hide the
    recurrent critical path (ACT works on wave B while wave A's cell update runs).
    """
    nc = bass.Bass(target_bir_lowering=False)

    xT = nc.declare_dram_parameter("xT", [T, D, BL], BF16, isOutput=False)
    wih_a = nc.declare_dram_parameter("wih_a", [D, R], F32, isOutput=False)
    wih_b = nc.declare_dram_parameter("wih_b", [R, G4], F32, isOutput=False)
    b_ih = nc.declare_dram_parameter("b_ih", [G4], F32, isOutput=False)
    whh_a = nc.declare_dram_parameter("whh_a", [D, R], F32, isOutput=False)
    whh_b = nc.declare_dram_parameter("whh_b", [R, G4], F32, isOutput=False)
    b_hh = nc.declare_dram_parameter("b_hh", [G4], F32, isOutput=False)
    outT = nc.declare_dram_parameter("outT", [T, H, BL], F32, isOutput=True)
    cT_out = nc.declare_dram_parameter("cT_out", [H, BL], F32, isOutput=True)

    BW = BL // waves
    # gate order in Wcat columns: i, f, g, o (j = gate*H + h)
    GATE_FUNCS = [AF.Sigmoid, AF.Sigmoid, AF.Tanh, AF.Sigmoid]

    with TileContext(nc) as tc:
        with (
            tc.tile_pool(name="singles", bufs=1) as singles,
            tc.tile_pool(name="xp", bufs=xbufs) as xp,
            tc.tile_pool(name="actp", bufs=abufs) as actp,
            tc.tile_pool(name="dvep", bufs=2) as dvep,
            tc.tile_pool(name="hp", bufs=2) as hp,
            tc.tile_pool(name="gpsum", bufs=(2 if waves == 1 else 3), space="PSUM") as gpsum,
            tc.tile_pool(name="pqpsum", bufs=2, space="PSUM") as pqpsum,
        ):
            # ---- weights / constants (bf16 stationaries, 128-wide for FWL) ----
            wa_sb = singles.tile([P, FC, P], BF16)
            nc.vector.memset(wa_sb, 0.0)
            nc.gpsimd.dma_start(
                out=wa_sb[:, :, 0:R], in_=wih_a.rearrange("(c p) r -> p c r", p=P)
            )
            ua_sb = singles.tile([P, HC, P], BF16)
            nc.vector.memset(ua_sb, 0.0)
            nc.gpsimd.dma_start(
                out=ua_sb[:, :, RP : RP + R],
                in_=whh_a.rearrange("(c p) r -> p c r", p=P),
            )

            wcat = singles.tile([KU, G4], BF16)
            nc.vector.memset(wcat, 0.0)
            nc.gpsimd.dma_start(out=wcat[0:R, :], in_=wih_b[:, :])
            nc.gpsimd.dma_start(out=wcat[RP : RP + R, :], in_=whh_b[:, :])
            nc.gpsimd.dma_start(out=wcat[64:65, :], in_=b_ih.rearrange("(o g) -> o g", o=1))
            nc.gpsimd.dma_start(out=wcat[65:66, :], in_=b_hh.rearrange("(o g) -> o g", o=1))

            u_all = singles.tile([KU, T, waves, BW], BF16)
            nc.vector.memset(u_all, 0.0)
            nc.vector.memset(u_all[64:66], 1.0)

            cT = singles.tile([P, waves, HC, BW], BF16)

            for _rep in range(reps):
                nc.vector.memset(cT, 0.0)
                h_prev = [None] * waves
                for t in range(T):
                    xts = []
                    for w in range(waves):
                        xt = xp.tile([P, FC, BW], BF16, tag=f"xt{w}")
                        nc.sync.dma_start(
                            out=xt,
                            in_=xT[t, :, w * BW : (w + 1) * BW].rearrange(
                                "(c p) b -> p c b", p=P
                            ),
                        )
                        xts.append(xt)
                    for w in range(waves):
                        xt = xts[w]
                        u_t = u_all[:, t, w, :]

                        if waves == 1:
                            pq_full = gpsum.tile([P, HC, BW], F32, tag="g")
                            pq = pq_full[:, 0, :]
                        else:
                            pq = pqpsum.tile([P, BW], F32, tag="pq")
                        n_mm = FC + (HC if t > 0 else 0)
                        k = 0
                        for fc in range(FC):
                            nc.tensor.matmul(
                                pq,
                                lhsT=wa_sb[:, fc, :],
                                rhs=xt[:, fc, :],
                                start=(k == 0),
                                stop=(k == n_mm - 1),
                            )
                            k += 1
                        if t > 0:
                            for fc in range(HC):
                                nc.tensor.matmul(
                                    pq,
                                    lhsT=ua_sb[:, fc, :],
                                    rhs=h_prev[w][:, fc, :],
                                    start=(k == 0),
                                    stop=(k == n_mm - 1),
                                )
                                k += 1
                        if merge_ucopy:
                            nc.vector.tensor_copy(u_t[0:QE, :], pq[0:QE, :])
                        else:
                            nc.vector.tensor_copy(u_t[0:RP, :], pq[0:RP, :])
                            if qcopy_act:
                                nc.scalar.copy(u_t[RP:QE, :], pq[RP:QE, :])
                            else:
                                nc.vector.tensor_copy(u_t[RP:QE, :], pq[RP:QE, :])

                        # ---- gates: one [128 j, 4, BW] PSUM tile per gate ----
                        acts = [None] * 4
                        for gi in (2, 0, 1, 3):  # g,i,f,o: DVE tail needs g,i first
                            a_t = actp.tile([P, HC, BW], BF16, tag=f"act{w}_{gi}")
                            acts[gi] = a_t
                            gp = gpsum.tile([P, HC, BW], F32, tag="g")
                            for sub in range(HC):
                                jc = gi * 4 + sub
                                nc.tensor.matmul(
                                    gp[:, sub, :],
                                    lhsT=wcat[:, jc * P : (jc + 1) * P],
                                    rhs=u_t,
                                    start=True,
                                    stop=True,
                                )
                            nc.scalar.activation(
                                a_t.rearrange("p c b -> p (c b)"),
                                gp.rearrange("p c b -> p (c b)"),
                                GATE_FUNCS[gi],
                            )

                        a_i, a_f, a_g, a_o = acts
                        cw = cT[:, w]

                        # ---- cell update, chunked so h releases early ----
                        t1 = dvep.tile([P, HC, BW], BF16, tag=f"t1{w}")
                        t2 = dvep.tile([P, HC, BW], BF16, tag=f"t2{w}")
                        tanc = dvep.tile([P, HC, BW], BF16, tag=f"tanc{w}")
                        h_t = hp.tile([P, HC, BW], BF16, tag=f"h{w}")
                        cs = HC // tanc_chunks
                        e_t2 = nc.gpsimd if t2_gps else nc.vector
                        e_t1 = nc.gpsimd if t1_gps else nc.vector
                        e_add = nc.gpsimd if add_gps else nc.vector
                        for ci in range(tanc_chunks):
                            sl = slice(ci * cs, (ci + 1) * cs)
                            e_t2.tensor_mul(t2[:, sl], a_i[:, sl], a_g[:, sl])
                            e_t1.tensor_mul(t1[:, sl], a_f[:, sl], cw[:, sl])
                            e_add.tensor_add(cw[:, sl], t1[:, sl], t2[:, sl])
                            nc.scalar.activation(
                                tanc[:, sl].rearrange("p c b -> p (c b)"),
                                cw[:, sl].rearrange("p c b -> p (c b)"),
                                AF.Tanh,
                            )
                            nc.vector.tensor_mul(h_t[:, sl], a_o[:, sl], tanc[:, sl])

                        nc.gpsimd.dma_start(
                            out=outT[t, :, w * BW : (w + 1) * BW].rearrange(
                                "(c p) b -> p c b", p=P
                            ),
                            in_=h_t,
                        )
                        h_prev[w] = h_t

            nc.gpsimd.dma_start(
                out=cT_out.rearrange("(c p) (w b) -> p w c b", p=P, w=waves),
                in_=cT,
            )

    _legalize_sync_waits(nc)
    return nc


_NC_CACHE = None


def _get_nc():
    global _NC_CACHE
    if _NC_CACHE is None:
        _NC_CACHE = build()
    return _NC_CACHE


def _make_in_maps(x, wih_a, wih_b, b_ih, whh_a, whh_b, b_hh):
    x = np.ascontiguousarray(np.asarray(x, dtype=np.float32))
    wih_a = np.asarray(wih_a, dtype=np.float32)
    wih_b = np.asarray(wih_b, dtype=np.float32)
    b_ih = np.asarray(b_ih, dtype=np.float32)
    whh_a = np.asarray(whh_a, dtype=np.float32)
    whh_b = np.asarray(whh_b, dtype=np.float32)
    b_hh = np.asarray(b_hh, dtype=np.float32)

    in_maps = []
    for c in range(NCORES):
        xs = x[c * BL : (c + 1) * BL]  # [BL, T, D]
        xT_np = np.ascontiguousarray(
            xs.transpose(1, 2, 0).astype(ml_dtypes.bfloat16)
        )  # [T, D, BL] bf16
        in_maps.append(
            {
                "xT": xT_np,
                "wih_a": wih_a,
                "wih_b": wih_b,
                "b_ih": b_ih,
                "whh_a": whh_a,
                "whh_b": whh_b,
                "b_hh": b_hh,
            }
        )
    return in_maps


def time_kernel(np_inputs, tmpdir=None):
    """Run once with NTFF tracing; returns (exec_time_ns, results_obj)."""
    nc = _get_nc()
    in_maps = _make_in_maps(**np_inputs)
    res = run_bass_kernel_spmd(
        nc, in_maps, list(range(NCORES)), trace=True, tmpdir=tmpdir
    )
    return res.exec_time_ns, res


def kernel(x, wih_a, wih_b, b_ih, whh_a, whh_b, b_hh):
    nc = _get_nc()
    in_maps = _make_in_maps(x, wih_a, wih_b, b_ih, whh_a, whh_b, b_hh)
    res = run_bass_kernel_spmd(nc, in_maps, list(range(NCORES)))

    outs = []
    cs = []
    for c in range(NCORES):
        oT = res.results[c]["outT"]  # [T, H, BL]
        outs.append(np.ascontiguousarray(oT.transpose(2, 0, 1)))  # [BL, T, H]
        cs.append(np.ascontiguousarray(res.results[c]["cT_out"].T))  # [BL, H]
    output = np.concatenate(outs, axis=0)
    c_final = np.concatenate(cs, axis=0)
    h_final = np.ascontiguousarray(output[:, -1, :])
    return (output, (h_final, c_final))


if __name__ == "__main__":
    # quick self-run with random data
    rng = np.random.default_rng(0)
    s_in = 1.0 / np.sqrt(D)
    inputs = {
        "x": rng.standard_normal((B, T, D), dtype=np.float32),
        "wih_a": (rng.standard_normal((D, R)) * s_in).astype(np.float32),
        "wih_b": (rng.standard_normal((R, G4)) * s_in).astype(np.float32),
        "b_ih": (rng.standard_normal((G4,)) * s_in).astype(np.float32),
        "whh_a": (rng.standard_normal((D, R)) * s_in).astype(np.float32),
        "whh_b": (rng.standard_normal((R, G4)) * s_in).astype(np.float32),
        "b_hh": (rng.standard_normal((G4,)) * s_in).astype(np.float32),
    }
    out, (h, c) = kernel(**inputs)
    print("out", out.shape, out.dtype, "h", h.shape, "c", c.shape)
